# revision 6
# baseline (speedup 1.0000x reference)
"""GAT layer kernel for 8 Trainium2 NeuronCores.

Strategy (edge-parallel, node-partitioned output; zero collectives):
  - Shard edges across the 8 cores by dst-node range: core c owns nodes
    [c*N/8, (c+1)*N/8) and receives exactly the edges pointing into them.
  - Every core computes the full h = x @ W on device (PE), augmented with
    s_src = h@a[:F], s_dst = h@a[F:] as extra columns, written to a DRAM
    table of 96-f32 rows (pairs of rows -> 768B gather elements so indices
    fit in int16 for the SWDGE dma_gather).
  - Per-core, local nodes are renumbered by descending in-degree (index
    bookkeeping on host).  The edges scheduled in "rank" order (k-th edge
    of every node) then form dense prefixes [0, cnt_r) of the renumbered
    node space: the segment-sum becomes plain dense vector adds into an
    SBUF accumulator - no scatter, no atomics, no collision races.
  - alpha normalization moves outside the segment sum:
      out[n] = elu( (sum_e alpha_exp_e * h[src_e]) / (alpha_sum[n]+1e-8) )
    The reference's global max subtraction cancels algebraically except
    inside the 1e-8 epsilon (relative effect < 1e-5); it is skipped.
  - Host work is only sharding, index bookkeeping, and output assembly.
"""
import sys
import numpy as np

try:
    import concourse.bacc as bacc
except ImportError:
    sys.path.insert(0, "/opt/trn_rl_repo")
    import concourse.bacc as bacc
import concourse.tile as tile
import concourse.mybir as mybir
from concourse import bass_utils
from concourse.masks import make_identity

C = 8                 # cores
GQ_TOK = 1024         # max tokens per dma_gather call (64-desc packet limit)
TROW = 96             # f32 per table row:  [h(64) | s_src | s_dst | pad]
PAIR = 2 * TROW       # f32 per gather element (768B)

F32 = mybir.dt.float32
I16 = mybir.dt.int16


def _wrap16(a):
    """[K] int16 -> [128, K//16]: token j at [j%16, j//16], replicated to
    the 8 gpsimd core groups."""
    w = np.ascontiguousarray(a.reshape(-1, 16).T)
    return np.tile(w, (8, 1))


def _prep(edge_index, edge_weight, N):
    """Shard + schedule (index bookkeeping only).

    Returns (sched, per_core_inputs, perms, tot, NLP)."""
    NL = N // C
    src = np.asarray(edge_index[0], dtype=np.int64)
    dst = np.asarray(edge_index[1], dtype=np.int64)
    w = np.asarray(edge_weight, dtype=np.float32)

    cores = []
    max_cnt = np.zeros(0, np.int64)
    for c in range(C):
        m = (dst >= c * NL) & (dst < (c + 1) * NL)
        s_c = src[m]
        d_c = dst[m] - c * NL
        w_c = w[m]
        deg = np.bincount(d_c, minlength=NL)
        perm = np.argsort(-deg, kind="stable")          # position -> natural
        order = np.argsort(d_c, kind="stable")          # edges grouped by dst
        starts = np.zeros(NL + 1, np.int64)
        starts[1:] = np.cumsum(deg)
        maxdeg = int(deg.max()) if deg.size else 0
        hist = np.bincount(deg, minlength=maxdeg + 2)
        cnt = NL - np.cumsum(hist)[:maxdeg + 1]          # #nodes with deg > r
        cnt = cnt[cnt > 0]
        cores.append(dict(s=s_c, w=w_c, perm=perm, order=order,
                          starts=starts, cnt=cnt))
        if len(cnt) > len(max_cnt):
            mc = np.zeros(len(cnt), np.int64)
            mc[:len(max_cnt)] = max_cnt
            max_cnt = mc
        max_cnt[:len(cnt)] = np.maximum(max_cnt[:len(cnt)], cnt)

    # shared schedule: per rank r, roundup(max_c cnt_r, 128) positions,
    # split into chunks of <= GQ_TOK tokens; chunk dest block = boff
    sched = []           # (K, slots, boff)
    rank_base = []       # token offset of each rank in the flat streams
    tot = 0
    for r in range(len(max_cnt)):
        c128 = int(-(-max_cnt[r] // 128) * 128)
        rank_base.append(tot)
        p = 0
        while p < c128:
            K = min(GQ_TOK, c128 - p)
            sched.append((K, K // 128, p // 128))
            p += K
        tot += c128

    NLP = -(-NL // 128) * 128     # padded node positions per core
    per_core = []
    for c in range(C):
        cc = cores[c]
        perm, order, starts, cnt = cc["perm"], cc["order"], cc["starts"], cc["cnt"]
        pair = np.zeros(tot, np.int16)
        par = np.zeros(tot, np.float32)
        wt = np.zeros(tot, np.float32)
        mask = np.zeros(tot, np.float32)
        for r in range(len(max_cnt)):
            n = int(cnt[r]) if r < len(cnt) else 0
            if n == 0:
                continue
            o = rank_base[r]
            eid = order[starts[perm[:n]] + r]
            sg = cc["s"][eid]
            pair[o:o + n] = (sg >> 1).astype(np.int16)
            par[o:o + n] = (sg & 1).astype(np.float32)
            wt[o:o + n] = cc["w"][eid]
            mask[o:o + n] = 1.0

        sidx_cols, meta_cols = [], []
        off = 0
        for (K, slots, boff) in sched:
            sidx_cols.append(_wrap16(pair[off:off + K]))
            def tl(a):
                return np.ascontiguousarray(a[off:off + K].reshape(slots, 128).T)
            meta_cols.append(np.stack([tl(par), tl(wt), tl(mask)],
                                      axis=2).reshape(128, slots * 3))
            off += K
        sidx = np.concatenate(sidx_cols, axis=1)
        meta = np.concatenate(meta_cols, axis=1)

        gnode = np.zeros(NLP, np.int64)
        gnode[:NL] = c * NL + perm
        pgidx_cols = []
        p = 0
        while p < NLP:
            K = min(GQ_TOK, NLP - p)
            pgidx_cols.append(_wrap16((gnode[p:p + K] >> 1).astype(np.int16)))
            p += K
        pgidx = np.concatenate(pgidx_cols, axis=1)
        ppar = np.ascontiguousarray(
            (gnode & 1).astype(np.float32).reshape(NLP // 128, 128).T)

        per_core.append(dict(sidx=sidx, meta=meta, pgidx=pgidx, ppar=ppar))

    return sched, per_core, [cores[c]["perm"] for c in range(C)], tot, NLP


_BUILD_CACHE = {}


def _build(N, F, O, sched, tot, NLP):
    key = (N, F, O, tuple(sched), tot, NLP)
    if key in _BUILD_CACHE:
        return _BUILD_CACHE[key]
    NB = NLP // 128           # node blocks per core
    TOT16 = tot // 16
    TOT128 = tot // 128
    nxt = -(-N // 128)        # x tiles

    nc = bacc.Bacc("TRN2", target_bir_lowering=False,
                   dynamic_dma_scratch_size=32768, num_swdge_queues=1)
    x_t = nc.dram_tensor("x", [N, F], F32, kind="ExternalInput")
    w_t = nc.dram_tensor("W", [F, O], F32, kind="ExternalInput")
    a_t = nc.dram_tensor("a", [2 * O], F32, kind="ExternalInput")
    sidx_t = nc.dram_tensor("sidx", [128, TOT16], I16, kind="ExternalInput")
    meta_t = nc.dram_tensor("meta", [128, TOT128 * 3], F32, kind="ExternalInput")
    pgidx_t = nc.dram_tensor("pgidx", [128, NLP // 16], I16, kind="ExternalInput")
    ppar_t = nc.dram_tensor("ppar", [128, NB], F32, kind="ExternalInput")
    out_t = nc.dram_tensor("out", [NLP, O], F32, kind="ExternalOutput")

    with tile.TileContext(nc) as tc:
        with (
            tc.tile_pool(name="persist", bufs=1) as pp,
            tc.tile_pool(name="dram", bufs=1, space="DRAM") as dp,
            tc.tile_pool(name="psum", bufs=2, space="PSUM") as psp,
            tc.tile_pool(name="work", bufs=3) as wp,
            tc.tile_pool(name="gpool", bufs=3) as gp,
        ):
            table = dp.tile([N, TROW], F32)
            tpair = table[:].rearrange("(p two) r -> p (two r)", two=2)

            # ---- phase A: W_aug = [W | W@a1 | W@a2] ----
            ident = pp.tile([128, 128], F32)
            make_identity(nc, ident[:])
            ws = pp.tile([128, O], F32)
            nc.sync.dma_start(ws[:], w_t[:])
            a1 = pp.tile([O, 1], F32)
            a2 = pp.tile([O, 1], F32)
            nc.sync.dma_start(a1[:], a_t[:O, None])
            nc.sync.dma_start(a2[:], a_t[O:, None])
            wtp = psp.tile([O, 128], F32, space="PSUM")
            nc.tensor.transpose(out=wtp[:], in_=ws[:], identity=ident[:])
            wts = pp.tile([O, 128], F32)
            nc.vector.tensor_copy(wts[:], wtp[:])
            vab = psp.tile([128, 2], F32, space="PSUM")
            nc.tensor.matmul(out=vab[:, 0:1], lhsT=wts[:], rhs=a1[:],
                             start=True, stop=True)
            nc.tensor.matmul(out=vab[:, 1:2], lhsT=wts[:], rhs=a2[:],
                             start=True, stop=True)
            waug = pp.tile([128, O + 2], F32)
            nc.vector.tensor_copy(waug[:, :O], ws[:])
            nc.vector.tensor_copy(waug[:, O:], vab[:])

            # ---- phase B: table rows [h | s_src | s_dst | pad] ----
            for t in range(nxt):
                r0 = t * 128
                nr = min(128, N - r0)
                xt = wp.tile([128, F], F32, tag="xt")
                nc.sync.dma_start(xt[:nr], x_t[r0:r0 + nr, :])
                xtp = psp.tile([F, 128], F32, space="PSUM", tag="xtp")
                nc.tensor.transpose(out=xtp[:, :nr], in_=xt[:nr],
                                    identity=ident[:nr, :nr])
                xts = wp.tile([F, 128], F32, tag="xts")
                nc.vector.tensor_copy(xts[:, :nr], xtp[:, :nr])
                hp = psp.tile([128, O + 2], F32, space="PSUM", tag="hp")
                nc.tensor.matmul(out=hp[:nr], lhsT=xts[:, :nr], rhs=waug[:],
                                 start=True, stop=True)
                hs = wp.tile([128, TROW], F32, tag="hs")
                nc.vector.tensor_copy(hs[:nr, :O + 2], hp[:nr])
                nc.vector.memset(hs[:nr, O + 2:], 0.0)
                nc.sync.dma_start(table[r0:r0 + nr, :], hs[:nr])

            # ---- phase B2: s_dst in permuted order ----
            pparS = pp.tile([128, NB], F32)
            nc.sync.dma_start(pparS[:], ppar_t[:])
            sdp = pp.tile([128, NB], F32)
            p = 0
            qn = 0
            while p < NLP:
                K = min(GQ_TOK, NLP - p)
                sl = K // 128
                b0 = p // 128
                pgi = wp.tile([128, GQ_TOK // 16], I16, tag="pgi")
                nc.sync.dma_start(pgi[:, :K // 16],
                                  pgidx_t[:, p // 16:(p + K) // 16])
                pg = gp.tile([128, GQ_TOK // 128, PAIR], F32, tag="pg")
                nc.gpsimd.dma_gather(
                    out_ap=pg[:, :sl, :], in_ap=tpair, idxs_ap=pgi[:, :K // 16],
                    num_idxs=K, num_idxs_reg=K, elem_size=PAIR, queue_num=0)
                lo = pg[:, :sl, O + 1]
                hi = pg[:, :sl, TROW + O + 1]
                tmp = wp.tile([128, GQ_TOK // 128], F32, tag="sdtmp")
                nc.vector.tensor_sub(tmp[:, :sl], hi, lo)
                nc.vector.tensor_mul(tmp[:, :sl], tmp[:, :sl],
                                     pparS[:, b0:b0 + sl])
                nc.vector.tensor_add(sdp[:, b0:b0 + sl], lo, tmp[:, :sl])
                p += K

            # ---- edge phase: dense rank accumulation ----
            accum = pp.tile([128, NB, O + 1], F32)
            nc.vector.memset(accum[:], 0.0)
            o16 = 0
            o128 = 0
            for j, (K, sl, boff) in enumerate(sched):
                si = wp.tile([128, GQ_TOK // 16], I16, tag="si")
                nc.sync.dma_start(si[:, :K // 16],
                                  sidx_t[:, o16:o16 + K // 16])
                mt = wp.tile([128, (GQ_TOK // 128) * 3], F32, tag="mt")
                nc.sync.dma_start(mt[:, :sl * 3],
                                  meta_t[:, o128 * 3:(o128 + sl) * 3])
                gt = gp.tile([128, GQ_TOK // 128, PAIR], F32, tag="gt")
                nc.gpsimd.dma_gather(
                    out_ap=gt[:, :sl, :], in_ap=tpair, idxs_ap=si[:, :K // 16],
                    num_idxs=K, num_idxs_reg=K, elem_size=PAIR, queue_num=0)
                mt3 = mt[:, :sl * 3].rearrange("p (s k) -> p s k", k=3)
                par = mt3[:, :, 0]
                wgt = mt3[:, :, 1]
                msk = mt3[:, :, 2]

                sel = wp.tile([128, GQ_TOK // 128, O + 1], F32, tag="sel")
                parb = par[:, :, None].to_broadcast([128, sl, O + 1])
                nc.vector.tensor_sub(sel[:, :sl, :],
                                     gt[:, :sl, TROW:TROW + O + 1],
                                     gt[:, :sl, :O + 1])
                nc.vector.tensor_mul(sel[:, :sl, :], sel[:, :sl, :], parb)
                nc.vector.tensor_add(sel[:, :sl, :], sel[:, :sl, :],
                                     gt[:, :sl, :O + 1])

                # e = s_src + s_dst ; alpha = exp(leaky(e) * w) * mask
                e = wp.tile([128, GQ_TOK // 128], F32, tag="e")
                nc.vector.tensor_add(e[:, :sl], sel[:, :sl, O],
                                     sdp[:, boff:boff + sl])
                lk = wp.tile([128, GQ_TOK // 128], F32, tag="lk")
                nc.vector.tensor_scalar(out=lk[:, :sl], in0=e[:, :sl],
                                        scalar1=0.0, scalar2=None,
                                        op0=mybir.AluOpType.min)
                nc.vector.tensor_scalar(out=e[:, :sl], in0=e[:, :sl],
                                        scalar1=0.0, scalar2=None,
                                        op0=mybir.AluOpType.max)
                nc.vector.tensor_scalar(out=lk[:, :sl], in0=lk[:, :sl],
                                        scalar1=0.2, scalar2=None,
                                        op0=mybir.AluOpType.mult)
                nc.vector.tensor_add(e[:, :sl], e[:, :sl], lk[:, :sl])
                nc.vector.tensor_mul(e[:, :sl], e[:, :sl], wgt)
                ax = wp.tile([128, GQ_TOK // 128], F32, tag="ax")
                nc.scalar.activation(ax[:, :sl], e[:, :sl],
                                     mybir.ActivationFunctionType.Exp)
                nc.vector.tensor_mul(ax[:, :sl], ax[:, :sl], msk)

                # msg = [h*axp | axp]; sel col O set to 1 first
                nc.vector.memset(sel[:, :sl, O], 1.0)
                nc.vector.tensor_mul(
                    sel[:, :sl, :],
                    sel[:, :sl, :],
                    ax[:, :sl, None].to_broadcast([128, sl, O + 1]))
                nc.vector.tensor_add(accum[:, boff:boff + sl, :],
                                     accum[:, boff:boff + sl, :],
                                     sel[:, :sl, :])
                o16 += K // 16
                o128 += sl

            # ---- final: out = elu(S / (alpha_sum + 1e-8)) ----
            rc = pp.tile([128, NB], F32)
            nc.vector.tensor_scalar(out=rc[:], in0=accum[:, :, O],
                                    scalar1=1e-8, scalar2=None,
                                    op0=mybir.AluOpType.add)
            nc.vector.reciprocal(rc[:], rc[:])
            ov = pp.tile([128, NB, O], F32)
            nc.vector.tensor_mul(ov[:], accum[:, :, :O],
                                 rc[:, :, None].to_broadcast([128, NB, O]))
            neg = pp.tile([128, NB, O], F32)
            nc.vector.tensor_scalar(out=neg[:], in0=ov[:], scalar1=0.0,
                                    scalar2=None, op0=mybir.AluOpType.min)
            nc.scalar.activation(neg[:], neg[:],
                                 mybir.ActivationFunctionType.Exp)
            nc.vector.tensor_scalar(out=ov[:], in0=ov[:], scalar1=0.0,
                                    scalar2=-1.0, op0=mybir.AluOpType.max,
                                    op1=mybir.AluOpType.add)
            nc.vector.tensor_add(ov[:], ov[:], neg[:])
            nc.sync.dma_start(
                out_t[:].rearrange("(b p) f -> p b f", p=128), ov[:])

    nc.compile()
    _BUILD_CACHE[key] = nc
    return nc


def kernel(x, edge_index, edge_weight, W, a):
    x = np.ascontiguousarray(np.asarray(x, dtype=np.float32))
    W = np.ascontiguousarray(np.asarray(W, dtype=np.float32))
    a = np.ascontiguousarray(np.asarray(a, dtype=np.float32))
    N, F = x.shape
    O = W.shape[1]
    NL = N // C

    sched, per_core, perms, tot, NLP = _prep(edge_index, edge_weight, N)
    nc = _build(N, F, O, sched, tot, NLP)

    in_maps = []
    for c in range(C):
        pc = per_core[c]
        in_maps.append({
            "x": x, "W": W, "a": a,
            "sidx": pc["sidx"], "meta": pc["meta"],
            "pgidx": pc["pgidx"], "ppar": pc["ppar"],
        })
    res = bass_utils.run_bass_kernel_spmd(nc, in_maps, core_ids=list(range(C)))

    out = np.empty((N, O), np.float32)
    for c in range(C):
        op = res.results[c]["out"]          # [NLP, O] in permuted order
        out[c * NL + perms[c]] = op[:NL]
    return out
